# revision 31
# baseline (speedup 1.0000x reference)
"""Trainium2 Bass kernel for nn_Adj (topk_masking).

Computes, per batch b:
    si   = x_b @ x_b^T                      (512, 512)
    th_i = 32nd largest value of row i
    adj  = (si >= th)                       row degree == 32 (no boundary ties)
    out  = adj / 32                         (D^-1/2 A D^-1/2 with D = 32 I)

Sharding: pure data-parallel over batch; core i handles batches [8i, 8i+8).
The host pre-transposes x so each core receives x_b^T (C-major), which is
what the PE needs for both matmul operands (si = lhsT.T @ rhs with
lhsT = rhs = x_b^T); no on-chip transposes of the input.

si is symmetric, so only the upper-triangular 128-row x 128-col blocks are
computed by matmul (62.5% of the FLOPs); the lower blocks are mirrored with
PE transposes. Top-32 per row via DVE max8/match_replace8; the final
mask+scale runs on the (otherwise idle) GPSIMD engine.
"""

import os
import sys

import numpy as np


def _import_concourse():
    try:
        import concourse.bass  # noqa: F401
        return
    except ImportError:
        pass
    for p in ("/opt/trn_rl_repo", "/root/.axon_site/_ro/trn_rl_repo"):
        if os.path.isdir(p) and p not in sys.path:
            sys.path.insert(0, p)
    import concourse.bass  # noqa: F401


B, N, C = 64, 512, 1024
K = 32
NCORES = 8
BPC = B // NCORES  # batches per core
P = 128            # SBUF partitions
KT = C // P        # contraction tiles per batch
MT = N // P        # output row tiles per batch
NEG = -1.0e30      # replacement sentinel, far below any |si| value

# "f32_tri": exact fp32 matmul on upper-triangular blocks + mirrored
# transposes. "f32r_full": full square matmul in f32r (PE full rate but
# ~11-bit input mantissa: ~300 boundary flips, rel err ~1.7e-2).
MODE = os.environ.get("ADJ_MODE", "f32_tri")


def _build_nc(mode=MODE):
    _import_concourse()
    import concourse.bacc as bacc
    import concourse.mybir as mybir
    from concourse.masks import make_identity
    from concourse.tile import TileContext

    tri = mode.endswith("_tri")
    fr = mybir.dt.float32r if mode.startswith("f32r") else mybir.dt.float32

    nc = bacc.Bacc("TRN2", target_bir_lowering=False)
    xt = nc.dram_tensor("xt", [BPC, C, N], fr, kind="ExternalInput")
    out = nc.dram_tensor("out", [BPC, N, N], mybir.dt.float32, kind="ExternalOutput")

    with TileContext(nc) as tc:
        with (
            tc.tile_pool(name="xtp", bufs=2) as xtp,
            tc.tile_pool(name="psp", bufs=4, space="PSUM") as psp,
            tc.tile_pool(name="tpp", bufs=3, space="PSUM") as tpp,
            tc.tile_pool(name="sip", bufs=8) as sip,
            tc.tile_pool(name="wrkp", bufs=4) as wrkp,
            tc.tile_pool(name="v8p", bufs=16) as v8p,
            tc.tile_pool(name="mp", bufs=8) as mp,
            tc.tile_pool(name="cstp", bufs=1) as cstp,
        ):
            ident = None
            if tri:
                ident = cstp.tile([P, P], mybir.dt.float32)
                make_identity(nc, ident)
            # PE warmup: ~5us of dummy matmuls while the first batch DMAs in,
            # so the HAM clock gate is fully open when real work starts.
            warm = cstp.tile([P, N], mybir.dt.float32)
            nc.gpsimd.memset(warm, 0.0)
            sig_bias = cstp.tile([P, 1], mybir.dt.float32)
            nc.gpsimd.memset(sig_bias, 0.25e9)
            wps = psp.tile([P, N], mybir.dt.float32, tag="ps")
            for w in range(4):
                nc.tensor.matmul(
                    wps,
                    lhsT=warm[:, :P],
                    rhs=warm,
                    start=True,
                    stop=True,
                )
            for b in range(BPC):
                xtb = xtp.tile([P, KT, N], fr)
                # Split each k-tile load across queues, issued from several
                # sequencers so descriptor generation is not serialized on
                # one engine. Batch 0 gates the whole pipeline start, so it
                # uses 4 issue streams; later batches prefetch with 2.
                h = N // 2
                if b == 0:
                    engs = [nc.sync, nc.gpsimd, nc.scalar]
                    for k in range(KT):
                        engs[(2 * k) % 3].dma_start(
                            out=xtb[:, k, :h],
                            in_=xt[b, k * P:(k + 1) * P, :h],
                        )
                        engs[(2 * k + 1) % 3].dma_start(
                            out=xtb[:, k, h:],
                            in_=xt[b, k * P:(k + 1) * P, h:],
                        )
                else:
                    for k in range(KT):
                        nc.sync.dma_start(
                            out=xtb[:, k, :h],
                            in_=xt[b, k * P:(k + 1) * P, :h],
                        )
                        nc.gpsimd.dma_start(
                            out=xtb[:, k, h:],
                            in_=xt[b, k * P:(k + 1) * P, h:],
                        )
                sis = []
                srcs = []
                for m in range(MT):
                    c0 = m * P if tri else 0  # first computed column
                    ps = psp.tile([P, N], mybir.dt.float32)
                    for k in range(KT):
                        nc.tensor.matmul(
                            ps[:, c0:],
                            lhsT=xtb[:, k, m * P:(m + 1) * P],
                            rhs=xtb[:, k, c0:],
                            start=(k == 0),
                            stop=(k == KT - 1),
                        )
                    si = sip.tile([P, N], mybir.dt.float32)
                    sis.append(si)
                    nc.scalar.copy(si[:, c0:], ps[:, c0:])
                    if tri:
                        for j in range(m):
                            # block (m, j) = block (j, m)^T
                            pst = tpp.tile([P, P], mybir.dt.float32)
                            nc.tensor.transpose(
                                pst,
                                in_=sis[j][:, m * P:(m + 1) * P],
                                identity=ident,
                            )
                            nc.scalar.copy(si[:, j * P:(j + 1) * P], pst)
                    # For the very first tile, the topk and mask read the
                    # PSUM directly — skipping the first ACT copy shaves the
                    # pipeline ramp.
                    srcs.append(ps if (b == 0 and m == 0) else si)
                # top-32 per row: 4 rounds of max8, removing each round's 8
                # winners; round 4's minimum is the 32nd largest. Tiles are
                # processed in interleaved pairs: the DVE queue is strict
                # in-order, so alternating two independent chains hides the
                # semaphore latency between dependent ops of one tile.
                for m0 in range(0, MT, MT):
                    pair = list(range(MT))
                    wrks = {m: wrkp.tile([P, N], mybir.dt.float32,
                                         name=f"wrk{m}", tag=f"wrk{m}")
                            for m in pair}
                    cur = {m: srcs[m] for m in pair}
                    v8s = {}
                    for r in range(4):
                        for m in pair:
                            v8s[m] = v8p.tile([P, 8], mybir.dt.float32,
                                              name=f"v8_{m}", tag=f"v8_{m}")
                            nc.vector.max(out=v8s[m], in_=cur[m])
                        if r < 3:
                            for m in pair:
                                nc.vector.match_replace(
                                    out=wrks[m], in_to_replace=v8s[m],
                                    in_values=cur[m], imm_value=NEG,
                                )
                                cur[m] = wrks[m]
                    for m in pair:
                        v8 = v8s[m]
                        msk = mp.tile([P, N], mybir.dt.float32)
                        if b == BPC - 1 and m >= MT - 2:
                            # Last tiles: single DVE op keeps the post-DVE
                            # tail short (the 4-op ACT chain would sit on the
                            # critical path after the final MAX8).
                            nc.vector.tensor_scalar(
                                out=msk, in0=srcs[m], scalar1=v8[:, 7:8],
                                scalar2=1.0 / K,
                                op0=mybir.AluOpType.is_ge,
                                op1=mybir.AluOpType.mult,
                            )
                        else:
                            # mask+scale on ACT (DVE is the binding engine):
                            #   s1 = sign(si - th)        (exact subtract)
                            #   s2 = sigmoid(1e9*s1+.25e9) -> {0,1} incl s1==0
                            #   msk = s2 / 32
                            thn = v8p.tile([P, 1], mybir.dt.float32)
                            nc.scalar.activation(
                                thn, v8[:, 7:8],
                                mybir.ActivationFunctionType.Copy,
                                scale=-1.0,
                            )
                            s1 = wrkp.tile([P, N], mybir.dt.float32)
                            nc.scalar.activation(
                                s1, srcs[m], mybir.ActivationFunctionType.Sign,
                                bias=thn,
                            )
                            s2 = wrkp.tile([P, N], mybir.dt.float32)
                            nc.scalar.activation(
                                s2, s1, mybir.ActivationFunctionType.Sigmoid,
                                scale=1.0e9, bias=sig_bias,
                            )
                            nc.scalar.activation(
                                msk, s2, mybir.ActivationFunctionType.Copy,
                                scale=1.0 / K,
                            )
                        # Split stores: a full 256KB mask on one ~23GB/s
                        # queue takes ~11us, which would sit in the kernel
                        # tail. Issued from the sync sequencer (the gpsimd
                        # sequencer backlogs on input-descriptor generation).
                        q = N // 4
                        if b == BPC - 1 and m >= MT - 2:
                            for qi in range(4):
                                nc.sync.dma_start(
                                    out=out[b, m * P:(m + 1) * P,
                                            qi * q:(qi + 1) * q],
                                    in_=msk[:, qi * q:(qi + 1) * q])
                        else:
                            nc.sync.dma_start(
                                out=out[b, m * P:(m + 1) * P, :h],
                                in_=msk[:, :h])
                            nc.sync.dma_start(
                                out=out[b, m * P:(m + 1) * P, h:],
                                in_=msk[:, h:])
    nc.compile()
    return nc


_NC_CACHE = {}


def _get_nc(mode=MODE):
    if mode not in _NC_CACHE:
        _NC_CACHE[mode] = _build_nc(mode)
    return _NC_CACHE[mode]


def _run(xt, mode=MODE, trace=False):
    """xt: (B, C, N) float32, batch-transposed input. Returns (results, out)."""
    _import_concourse()
    from concourse.bass_utils import run_bass_kernel_spmd

    nc = _get_nc(mode)
    in_maps = [
        {"xt": np.ascontiguousarray(xt[i * BPC:(i + 1) * BPC])}
        for i in range(NCORES)
    ]
    res = run_bass_kernel_spmd(nc, in_maps, core_ids=list(range(NCORES)),
                               trace=trace)
    out = np.concatenate([res.results[i]["out"] for i in range(NCORES)], axis=0)
    return res, out


def kernel(x):
    x = np.asarray(x, dtype=np.float32)
    xt = np.ascontiguousarray(x.transpose(0, 2, 1))  # (B, C, N)
    _, out = _run(xt)
    return out


# revision 33
# speedup vs baseline: 1.0188x; 1.0188x over previous
"""Trainium2 Bass kernel for nn_Adj (topk_masking).

Computes, per batch b:
    si   = x_b @ x_b^T                      (512, 512)
    th_i = 32nd largest value of row i
    adj  = (si >= th)                       row degree == 32 (no boundary ties)
    out  = adj / 32                         (D^-1/2 A D^-1/2 with D = 32 I)

Sharding: pure data-parallel over batch; core i handles batches [8i, 8i+8).
The host pre-transposes x so each core receives x_b^T (C-major), which is
what the PE needs for both matmul operands (si = lhsT.T @ rhs with
lhsT = rhs = x_b^T); no on-chip transposes of the input.

si is symmetric, so only the upper-triangular 128-row x 128-col blocks are
computed by matmul (62.5% of the FLOPs); the lower blocks are mirrored with
PE transposes. Top-32 per row via DVE max8/match_replace8; the final
mask+scale runs as a saturated-sigmoid chain on the ACT engine.
"""

import os
import sys

import numpy as np


def _import_concourse():
    try:
        import concourse.bass  # noqa: F401
        return
    except ImportError:
        pass
    for p in ("/opt/trn_rl_repo", "/root/.axon_site/_ro/trn_rl_repo"):
        if os.path.isdir(p) and p not in sys.path:
            sys.path.insert(0, p)
    import concourse.bass  # noqa: F401


B, N, C = 64, 512, 1024
K = 32
NCORES = 8
BPC = B // NCORES  # batches per core
P = 128            # SBUF partitions
KT = C // P        # contraction tiles per batch
MT = N // P        # output row tiles per batch
NEG = -1.0e30      # replacement sentinel, far below any |si| value

# "f32_tri": exact fp32 matmul on upper-triangular blocks + mirrored
# transposes. "f32r_full": full square matmul in f32r (PE full rate but
# ~11-bit input mantissa: ~300 boundary flips, rel err ~1.7e-2).
MODE = os.environ.get("ADJ_MODE", "f32_tri")


def _build_nc(mode=MODE):
    _import_concourse()
    import concourse.bacc as bacc
    import concourse.mybir as mybir
    from concourse.masks import make_identity
    from concourse.tile import TileContext

    tri = mode.endswith("_tri")
    fr = mybir.dt.float32r if mode.startswith("f32r") else mybir.dt.float32

    nc = bacc.Bacc("TRN2", target_bir_lowering=False)
    xt = nc.dram_tensor("xt", [BPC, C, N], fr, kind="ExternalInput")
    out = nc.dram_tensor("out", [BPC, N, N], mybir.dt.float32, kind="ExternalOutput")

    with TileContext(nc) as tc:
        with (
            tc.tile_pool(name="xtp", bufs=2) as xtp,
            tc.tile_pool(name="psp", bufs=4, space="PSUM") as psp,
            tc.tile_pool(name="tpp", bufs=3, space="PSUM") as tpp,
            tc.tile_pool(name="sip", bufs=8) as sip,
            tc.tile_pool(name="wrkp", bufs=6) as wrkp,
            tc.tile_pool(name="v8p", bufs=16) as v8p,
            tc.tile_pool(name="mp", bufs=8) as mp,
            tc.tile_pool(name="cstp", bufs=1) as cstp,
        ):
            ident = None
            if tri:
                ident = cstp.tile([P, P], mybir.dt.float32)
                make_identity(nc, ident)
            # PE warmup: ~5us of dummy matmuls while the first batch DMAs in,
            # so the HAM clock gate is fully open when real work starts.
            warm = cstp.tile([P, N], mybir.dt.float32)
            nc.gpsimd.memset(warm, 0.0)
            sig_bias = cstp.tile([P, 1], mybir.dt.float32)
            nc.gpsimd.memset(sig_bias, 0.25e9)
            wps = psp.tile([P, N], mybir.dt.float32, tag="ps")
            for w in range(4):
                nc.tensor.matmul(
                    wps,
                    lhsT=warm[:, :P],
                    rhs=warm,
                    start=True,
                    stop=True,
                )
            for b in range(BPC):
                xtb = xtp.tile([P, KT, N], fr)
                # Split each k-tile load across queues, issued from several
                # sequencers so descriptor generation is not serialized on
                # one engine. Batch 0 gates the whole pipeline start, so it
                # uses 4 issue streams; later batches prefetch with 2.
                h = N // 2
                if b == 0:
                    engs = [nc.sync, nc.gpsimd, nc.scalar]
                    for k in range(KT):
                        engs[(2 * k) % 3].dma_start(
                            out=xtb[:, k, :h],
                            in_=xt[b, k * P:(k + 1) * P, :h],
                        )
                        engs[(2 * k + 1) % 3].dma_start(
                            out=xtb[:, k, h:],
                            in_=xt[b, k * P:(k + 1) * P, h:],
                        )
                else:
                    for k in range(KT):
                        nc.sync.dma_start(
                            out=xtb[:, k, :h],
                            in_=xt[b, k * P:(k + 1) * P, :h],
                        )
                        nc.gpsimd.dma_start(
                            out=xtb[:, k, h:],
                            in_=xt[b, k * P:(k + 1) * P, h:],
                        )
                sis = []
                srcs = []
                for m in range(MT):
                    c0 = m * P if tri else 0  # first computed column
                    ps = psp.tile([P, N], mybir.dt.float32)
                    for k in range(KT):
                        nc.tensor.matmul(
                            ps[:, c0:],
                            lhsT=xtb[:, k, m * P:(m + 1) * P],
                            rhs=xtb[:, k, c0:],
                            start=(k == 0),
                            stop=(k == KT - 1),
                        )
                    si = sip.tile([P, N], mybir.dt.float32)
                    sis.append(si)
                    nc.scalar.copy(si[:, c0:], ps[:, c0:])
                    if tri:
                        for j in range(m):
                            # block (m, j) = block (j, m)^T
                            pst = tpp.tile([P, P], mybir.dt.float32)
                            nc.tensor.transpose(
                                pst,
                                in_=sis[j][:, m * P:(m + 1) * P],
                                identity=ident,
                            )
                            nc.scalar.copy(si[:, j * P:(j + 1) * P], pst)
                    # For the very first tile, the topk and mask read the
                    # PSUM directly — skipping the first ACT copy shaves the
                    # pipeline ramp.
                    srcs.append(ps if (b == 0 and m == 0) else si)
                # top-32 per row: 4 rounds of max8, removing each round's 8
                # winners; round 4's minimum is the 32nd largest. Tiles are
                # processed in interleaved pairs: the DVE queue is strict
                # in-order, so alternating two independent chains hides the
                # semaphore latency between dependent ops of one tile.
                # Batch 0 ramp: run tiles 0 and 1 solo (tile 0 reads PSUM
                # directly and keeps DVE busy while tile 1's si is being
                # assembled); later batches use interleaved pairs.
                groups = ([[0], [1], [2, 3]] if b == 0
                          else [[0, 1], [2, 3]])
                for pair in groups:
                    wrks = {m: wrkp.tile([P, N], mybir.dt.float32,
                                         name=f"wrk{m % 2}", tag=f"wrk{m % 2}")
                            for m in pair}
                    cur = {m: srcs[m] for m in pair}
                    v8s = {}
                    for r in range(4):
                        for m in pair:
                            v8s[m] = v8p.tile([P, 8], mybir.dt.float32,
                                              name=f"v8_{m % 2}", tag=f"v8_{m % 2}")
                            nc.vector.max(out=v8s[m], in_=cur[m])
                        if r < 3:
                            for m in pair:
                                nc.vector.match_replace(
                                    out=wrks[m], in_to_replace=v8s[m],
                                    in_values=cur[m], imm_value=NEG,
                                )
                                cur[m] = wrks[m]
                    for m in pair:
                        v8 = v8s[m]
                        msk = mp.tile([P, N], mybir.dt.float32)
                        if b == BPC - 1 and m >= MT - 2:
                            # Last tiles: single DVE op keeps the post-DVE
                            # tail short (the 4-op ACT chain would sit on the
                            # critical path after the final MAX8).
                            nc.vector.tensor_scalar(
                                out=msk, in0=srcs[m], scalar1=v8[:, 7:8],
                                scalar2=1.0 / K,
                                op0=mybir.AluOpType.is_ge,
                                op1=mybir.AluOpType.mult,
                            )
                        else:
                            # mask+scale on ACT (DVE is the binding engine):
                            #   s1 = sign(si - th)        (exact subtract)
                            #   s2 = sigmoid(1e9*s1+.25e9) -> {0,1} incl s1==0
                            #   msk = s2 / 32
                            thn = v8p.tile([P, 1], mybir.dt.float32)
                            nc.scalar.activation(
                                thn, v8[:, 7:8],
                                mybir.ActivationFunctionType.Copy,
                                scale=-1.0,
                            )
                            s1 = wrkp.tile([P, N], mybir.dt.float32)
                            nc.scalar.activation(
                                s1, srcs[m], mybir.ActivationFunctionType.Sign,
                                bias=thn,
                            )
                            s2 = wrkp.tile([P, N], mybir.dt.float32)
                            nc.scalar.activation(
                                s2, s1, mybir.ActivationFunctionType.Sigmoid,
                                scale=1.0e9, bias=sig_bias,
                            )
                            nc.scalar.activation(
                                msk, s2, mybir.ActivationFunctionType.Copy,
                                scale=1.0 / K,
                            )
                        # Split stores: a full 256KB mask on one ~23GB/s
                        # queue takes ~11us, which would sit in the kernel
                        # tail. Issued from the sync sequencer (the gpsimd
                        # sequencer backlogs on input-descriptor generation).
                        q = N // 4
                        if b == BPC - 1 and m >= MT - 2:
                            for qi in range(4):
                                nc.sync.dma_start(
                                    out=out[b, m * P:(m + 1) * P,
                                            qi * q:(qi + 1) * q],
                                    in_=msk[:, qi * q:(qi + 1) * q])
                        else:
                            nc.sync.dma_start(
                                out=out[b, m * P:(m + 1) * P, :h],
                                in_=msk[:, :h])
                            nc.sync.dma_start(
                                out=out[b, m * P:(m + 1) * P, h:],
                                in_=msk[:, h:])
    nc.compile()
    return nc


_NC_CACHE = {}


def _get_nc(mode=MODE):
    if mode not in _NC_CACHE:
        _NC_CACHE[mode] = _build_nc(mode)
    return _NC_CACHE[mode]


def _run(xt, mode=MODE, trace=False):
    """xt: (B, C, N) float32, batch-transposed input. Returns (results, out)."""
    _import_concourse()
    from concourse.bass_utils import run_bass_kernel_spmd

    nc = _get_nc(mode)
    in_maps = [
        {"xt": np.ascontiguousarray(xt[i * BPC:(i + 1) * BPC])}
        for i in range(NCORES)
    ]
    res = run_bass_kernel_spmd(nc, in_maps, core_ids=list(range(NCORES)),
                               trace=trace)
    out = np.concatenate([res.results[i]["out"] for i in range(NCORES)], axis=0)
    return res, out


def kernel(x):
    x = np.asarray(x, dtype=np.float32)
    xt = np.ascontiguousarray(x.transpose(0, 2, 1))  # (B, C, N)
    _, out = _run(xt)
    return out
